# revision 29
# baseline (speedup 1.0000x reference)
"""Trainium2 Bass kernel for nn_AttackRMultiHeadAttention.

Math (per batch b, head h), matching the reference:
    q = x @ Wq + bq ; k = x @ Wk + bk ; v = x @ Wv + bv         (per-head slices)
    scores = q @ k^T
    z  = qo_i + ko_j + order_b        (qo = q @ ow1, ko = k @ ow2)
    w  = qd_i + kd_j + dist_b         (qd = q @ dw1, kd = k @ dw2)
    error_order    = -softplus(-z) - z * tril(i>=j)    [exact rewrite of
                      log(sigmoid(z))*triu + log(1-sigmoid(z))*(1-triu)]
    error_distance = -0.5 * s^2 * (g - w)^2,  g = log(1+|i-j|)
    adj = scores + error_order + error_distance
    attention_probs        = softmax(adj / 8)
    origin_attention_probs = softmax(scores / 8)

softplus(-z) is computed as ln(1 + exp(-qo')*exp(-ko)): the exp of a rank-1
term is an outer product of two exp'd vectors, built on the TensorEngine, so
the only full-tile ACT ops per [128,1024] tile are {Ln, Exp, Exp} - all in the
single activation-table set natural_log_exp_and_others.

Sharding: B x H = 16 head-units over 8 cores; core c takes batch c//4 and the
128-wide column slice c%4 of the QKV projections (2 heads). g / tril are
Toeplitz, so each core holds one [128, 1920] band and every i-tile's [128,1024]
constant block is a free-dim slice of it.

Softmax runs without the row-max subtraction: |adj|/8 is bounded by a few tens
for this model family (weights ~0.02 scale), far inside fp32 exp range.
"""

import os
import sys
import numpy as np

for _p in ("/opt/trn_rl_repo", "/opt/pypackages"):
    if _p not in sys.path:
        sys.path.insert(0, _p)

B, S, HID, H = 2, 1024, 512, 8
D = HID // H            # 64
NCORES = 8
HPC = 2                 # heads per core
CPW = HPC * D           # 128 projection columns per core
NT = S // 128           # 8 row tiles of 128
C_SM = 0.125            # 1/sqrt(D) = 1/8 softmax scale
BANDW = 1920            # 128 + 1024 + 768: covers j-i in [-1023, 1023]

_CACHE = {}
LAST_PROFILE = {}


def _build_nc():
    import concourse.bacc as bacc
    import concourse.mybir as mybir
    import concourse.tile as tile
    from concourse.masks import make_identity

    f32 = mybir.dt.float32
    f32r = mybir.dt.float32r
    AF = mybir.ActivationFunctionType
    OP = mybir.AluOpType

    nc = bacc.Bacc("TRN2", target_bir_lowering=False, debug=False,
                   enable_asserts=False)

    # ---------------- DRAM I/O ----------------
    x_d = nc.dram_tensor("x", [S, HID], f32, kind="ExternalInput")
    w_d = {nm: nc.dram_tensor(f"w{nm}", [HID, CPW], f32, kind="ExternalInput")
           for nm in ("q", "k", "v")}
    bqkv_d = nc.dram_tensor("bqkv", [1, 3 * CPW], f32, kind="ExternalInput")
    qsw_d = nc.dram_tensor("qsw", [128, 2], f32, kind="ExternalInput")
    ksw_d = nc.dram_tensor("ksw", [128, 2], f32, kind="ExternalInput")
    bg_d = nc.dram_tensor("bandg", [128, BANDW], f32, kind="ExternalInput")
    bl_d = nc.dram_tensor("bandl", [128, 128], f32, kind="ExternalInput")
    bias2_d = nc.dram_tensor("bias2", [1, 2], f32, kind="ExternalInput")

    mq_d = nc.dram_tensor("mq", [S, CPW], f32, kind="ExternalOutput")
    mk_d = nc.dram_tensor("mk", [S, CPW], f32, kind="ExternalOutput")
    vo_d = nc.dram_tensor("vo", [S, CPW], f32, kind="ExternalOutput")
    pa_d = nc.dram_tensor("pa", [HPC, S, S], f32, kind="ExternalOutput")
    po_d = nc.dram_tensor("po", [HPC, S, S], f32, kind="ExternalOutput")

    USE_F32R = False  # walrus requires fp32r-rounded producers; fp32 is exact
                      # and the PE is far from the bottleneck here.

    def r(ap):
        return ap.bitcast(f32r) if USE_F32R else ap

    with tile.TileContext(nc) as tc:
        with tc.tile_pool(name="const", bufs=1) as cp, \
             tc.tile_pool(name="mout", bufs=3) as mp, \
             tc.tile_pool(name="head", bufs=1) as hp, \
             tc.tile_pool(name="work", bufs=3) as wp, \
             tc.tile_pool(name="psum", bufs=4, space="PSUM") as pp:

            # ---------------- constants / inputs ----------------
            ident = cp.tile([128, 128], f32, name="ident")
            make_identity(nc, ident)
            bg = cp.tile([128, BANDW], f32, name="bg")
            nc.sync.dma_start(out=bg, in_=bg_d.ap())
            bl = cp.tile([128, 128], f32, name="bl")
            nc.sync.dma_start(out=bl, in_=bl_d.ap())
            b2 = cp.tile([1, 2], f32, name="b2")
            nc.sync.dma_start(out=b2, in_=bias2_d.ap())
            qsw = cp.tile([128, 2], f32, name="qsw")
            nc.sync.dma_start(out=qsw, in_=qsw_d.ap())
            ksw = cp.tile([128, 2], f32, name="ksw")
            nc.sync.dma_start(out=ksw, in_=ksw_d.ap())
            bqkv = cp.tile([1, 3 * CPW], f32, name="bqkv")
            nc.sync.dma_start(out=bqkv, in_=bqkv_d.ap())
            ones = cp.tile([1, 512], f32, name="ones")
            nc.vector.memset(ones, 1.0)

            wsb = {}
            for nm in ("q", "k", "v"):
                t = cp.tile([128, 4, CPW], f32, name=f"w{nm}")
                nc.sync.dma_start(
                    out=t, in_=w_d[nm].ap().rearrange("(c p) n -> p c n", p=128))
                wsb[nm] = t

            TT = {}
            with tc.tile_pool(name="xtmp", bufs=1) as xp:
                xall = xp.tile([128, NT, HID], f32, name="xall")
                nc.sync.dma_start(
                    out=xall, in_=x_d.ap().rearrange("(t p) k -> p t k", p=128))

                # ------------ phase 1: xT = x^T (PE transposes) -------------
                xT = xp.tile([128, 4, S], f32, name="xT")  # [k%128, k//128, i]
                for kc in range(4):
                    ps = pp.tile([128, 1024], f32, tag="ps", name="ps_x")
                    for t in range(NT):
                        nc.tensor.transpose(
                            ps[:, 128 * t:128 * (t + 1)],
                            xall[:, t, 128 * kc:128 * (kc + 1)], ident)
                    nc.any.tensor_copy(xT[:, kc, :], ps)

                # ------------ phase 2: qT/kT/vT projections ------------------
                # XT[c, i] = sum_k W[k, c] x[i, k] + b[c]
                for wi, nm in enumerate(("q", "k", "v")):
                    ps = pp.tile([128, 1024], f32, tag="ps", name=f"ps_p{nm}")
                    for nh in range(2):
                        sl = slice(512 * nh, 512 * (nh + 1))
                        for kc in range(4):
                            nc.tensor.matmul(ps[:, sl], r(wsb[nm][:, kc, :]),
                                             r(xT[:, kc, sl]),
                                             start=(kc == 0), stop=False)
                        nc.tensor.matmul(
                            ps[:, sl], r(bqkv[:, CPW * wi:CPW * (wi + 1)]),
                            r(ones[:, :512]), start=False, stop=True)
                    sb = cp.tile([128, S], f32, name=f"T{nm}")
                    nc.any.tensor_copy(sb, ps)
                    TT[nm] = sb

            # ---------------- phase 3: mixed_q / mixed_k / v outputs ---------
            for nm, od in (("q", mq_d), ("k", mk_d), ("v", vo_d)):
                for t in range(NT):
                    ps = pp.tile([128, 128], f32, tag="ps", name="ps_m")
                    nc.tensor.transpose(ps, TT[nm][:, 128 * t:128 * (t + 1)],
                                        ident)
                    mt = mp.tile([128, 128], f32, name="mt")
                    nc.any.tensor_copy(mt, ps)
                    nc.sync.dma_start(out=od.ap()[128 * t:128 * (t + 1), :],
                                      in_=mt)

            # ---------------- phase 4: attention per head --------------------
            for h in range(HPC):
                hb = D * h
                hsl = slice(hb, hb + D)

                # row vectors [1, 1024] each: qo', qd', ko, kd
                # (separate M=1 matmuls: PSUM reads must start at partition 0)
                qrow_o = hp.tile([1, 1024], f32, name="qrow_o")
                qrow_d = hp.tile([1, 1024], f32, name="qrow_d")
                krow_o = hp.tile([1, 1024], f32, name="krow_o")
                krow_d = hp.tile([1, 1024], f32, name="krow_d")
                for col, wt, src, dst, bia in (
                        (0, qsw, "q", qrow_o, b2[:, 0:1]),
                        (1, qsw, "q", qrow_d, b2[:, 1:2]),
                        (0, ksw, "k", krow_o, None),
                        (1, ksw, "k", krow_d, None)):
                    ps = pp.tile([1, 1024], f32, tag="ps", name="ps_rv")
                    for nh in range(2):
                        sl = slice(512 * nh, 512 * (nh + 1))
                        nc.tensor.matmul(ps[:, sl], r(wt[hsl, col:col + 1]),
                                         r(TT[src][hsl, sl]),
                                         start=True, stop=True)
                    if bia is None:
                        nc.any.tensor_copy(dst, ps)
                    else:
                        nc.vector.tensor_scalar_add(dst, ps, bia)

                # exp(-qo'), exp(-ko) rows for the softplus outer product
                evq = hp.tile([1, 1024], f32, name="evq")
                nc.scalar.activation(evq, qrow_o, AF.Exp, scale=-1.0)
                evk = hp.tile([1, 1024], f32, name="evk")
                nc.scalar.activation(evk, krow_o, AF.Exp, scale=-1.0)

                # broadcast ko / kd across partitions (rank-1 with ones)
                kob = hp.tile([128, S], f32, name="kob")
                kdb = hp.tile([128, S], f32, name="kdb")
                for srow, dst in ((krow_o, kob), (krow_d, kdb)):
                    ps = pp.tile([128, 1024], f32, tag="ps", name="ps_bc")
                    for nh in range(2):
                        sl = slice(512 * nh, 512 * (nh + 1))
                        nc.tensor.matmul(ps[:, sl], r(ones[:, :128]),
                                         r(srow[:, sl]),
                                         start=True, stop=True)
                    nc.any.tensor_copy(dst, ps)

                # q columns: [128, 1] per i-tile per vec, packed [128, 16]
                psc = pp.tile([128, 16], f32, tag="ps", name="ps_c")
                for t in range(NT):
                    nc.tensor.transpose(psc[:, 2 * t:2 * t + 1],
                                        qrow_o[:, 128 * t:128 * (t + 1)],
                                        ident[0:1, 0:1])
                    nc.tensor.transpose(psc[:, 2 * t + 1:2 * t + 2],
                                        qrow_d[:, 128 * t:128 * (t + 1)],
                                        ident[0:1, 0:1])
                qcols = hp.tile([128, 16], f32, name="qcols")
                nc.any.tensor_copy(qcols, psc)

                for t in range(NT):
                    off = 896 - 128 * t
                    gsl = bg[:, off:off + 1024]
                    ldiag = bl   # [128,128] tril mask of the diagonal block
                    qo_c = qcols[:, 2 * t:2 * t + 1]
                    qd_c = qcols[:, 2 * t + 1:2 * t + 2]
                    dsl = slice(128 * t, 128 * (t + 1))   # diagonal block cols

                    # scores and exp(-z) outer product
                    pss = pp.tile([128, 1024], f32, tag="ps", name="ps_s")
                    pst = pp.tile([128, 1024], f32, tag="ps", name="ps_t")
                    for nh in range(2):
                        sl = slice(512 * nh, 512 * (nh + 1))
                        nc.tensor.matmul(
                            pss[:, sl], r(TT["q"][hsl, 128 * t:128 * (t + 1)]),
                            r(TT["k"][hsl, sl]), start=True, stop=True)
                        nc.tensor.matmul(
                            pst[:, sl], r(evq[:, 128 * t:128 * (t + 1)]),
                            r(evk[:, sl]), start=True, stop=True)

                    # so = softplus(-z) = ln(1 + exp(-z))
                    so = wp.tile([128, 1024], f32, name="so")
                    nc.scalar.activation(so, pst, AF.Ln, bias=1.0, scale=1.0)
                    # dd = a*(g - w) = (ag - aqd') - akd   (a = |s|*sqrt(0.5)
                    # pre-folded into band_g / dist weights host-side), then
                    # ed2 = dd^2 so that error_distance = -ed2.
                    dd = wp.tile([128, 1024], f32, name="dd")
                    nc.vector.scalar_tensor_tensor(
                        dd, gsl, qd_c, kdb, OP.subtract, OP.subtract)
                    ed2 = wp.tile([128, 1024], f32, name="ed2")
                    nc.gpsimd.tensor_tensor(ed2, dd, dd, OP.mult)
                    # adj = scores - so - ed2 - z*tril   (z-term chunked:
                    # full columns below the diagonal block, masked on it)
                    adj = wp.tile([128, 1024], f32, name="adj")
                    nc.vector.tensor_sub(adj, pss, so)
                    nc.gpsimd.tensor_tensor(adj, adj, ed2, OP.subtract)
                    if t > 0:
                        lo = slice(0, 128 * t)
                        nc.vector.scalar_tensor_tensor(
                            adj[:, lo], adj[:, lo], qo_c, kob[:, lo],
                            OP.subtract, OP.subtract)
                    zd = wp.tile([128, 128], f32, name="zd")
                    nc.vector.scalar_tensor_tensor(
                        zd, kob[:, dsl], qo_c, ldiag, OP.add, OP.mult)
                    nc.vector.tensor_sub(adj[:, dsl], adj[:, dsl], zd)

                    # softmaxes (no max-subtraction; values are small)
                    ra = wp.tile([128, 1], f32, name="ra")
                    nc.scalar.activation(adj, adj, AF.Exp, scale=C_SM,
                                         accum_out=ra)
                    eo = wp.tile([128, 1024], f32, name="eo")
                    ro = wp.tile([128, 1], f32, name="ro")
                    nc.scalar.activation(eo, pss, AF.Exp, scale=C_SM,
                                         accum_out=ro)
                    nc.vector.reciprocal(ra, ra)
                    nc.vector.reciprocal(ro, ro)
                    nc.vector.tensor_scalar_mul(adj, adj, ra)
                    nc.scalar.activation(eo, eo, AF.Copy, scale=ro)
                    nc.sync.dma_start(
                        out=pa_d.ap()[h, 128 * t:128 * (t + 1), :], in_=adj)
                    nc.sync.dma_start(
                        out=po_d.ap()[h, 128 * t:128 * (t + 1), :], in_=eo)
    nc.compile()
    return nc


def _get_nc():
    if "nc" not in _CACHE:
        _CACHE["nc"] = _build_nc()
    return _CACHE["nc"]


def _get_runner():
    """Cached sharded PJRT executable over 8 cores (mirrors
    bass2jax.run_bass_via_pjrt but reusable across calls and without
    donation, so the NEFF can be re-executed for timing)."""
    if "runner" in _CACHE:
        return _CACHE["runner"]
    import jax
    from jax.sharding import Mesh, PartitionSpec, NamedSharding
    try:
        from jax.experimental.shard_map import shard_map
    except ImportError:
        from jax.shard_map import shard_map  # newer jax
    from concourse import bass2jax, mybir

    nc = _get_nc()
    bass2jax.install_neuronx_cc_hook()

    partition_name = (nc.partition_id_tensor.name
                      if nc.partition_id_tensor else None)
    in_names, out_names, out_avals, zero_outs = [], [], [], []
    for alloc in nc.m.functions[0].allocations:
        if not isinstance(alloc, mybir.MemoryLocationSet):
            continue
        name = alloc.memorylocations[0].name
        if alloc.kind == "ExternalInput":
            if name != partition_name:
                in_names.append(name)
        elif alloc.kind == "ExternalOutput":
            shape = tuple(alloc.tensor_shape)
            dtype = mybir.dt.np(alloc.dtype)
            out_names.append(name)
            out_avals.append(jax.core.ShapedArray(shape, dtype))
            zero_outs.append(np.zeros(shape, dtype))
    n_params = len(in_names)
    n_outs = len(out_names)
    bind_in_names = tuple(in_names + out_names +
                          ([partition_name] if partition_name else []))

    def _body(*args):
        operands = list(args)
        if partition_name is not None:
            operands.append(bass2jax.partition_id_tensor())
        outs = bass2jax._bass_exec_p.bind(
            *operands,
            out_avals=tuple(out_avals),
            in_names=bind_in_names,
            out_names=tuple(out_names),
            lowering_input_output_aliases=(),
            sim_require_finite=True,
            sim_require_nnan=True,
            nc=nc,
        )
        return tuple(outs)

    devices = jax.devices()[:NCORES]
    mesh = Mesh(np.asarray(devices), ("core",))
    specs = (PartitionSpec("core"),)
    fn = jax.jit(
        shard_map(_body, mesh=mesh, in_specs=specs * (n_params + n_outs),
                  out_specs=specs * n_outs, check_rep=False),
        keep_unused=True)
    sharding = NamedSharding(mesh, PartitionSpec("core"))
    runner = {
        "fn": fn, "jax": jax, "sharding": sharding,
        "in_names": in_names, "out_names": out_names,
        "out_avals": out_avals, "zero_outs": zero_outs,
    }
    _CACHE["runner"] = runner
    return runner


def _run_pjrt(in_maps):
    import time as _time
    rn = _get_runner()
    jax = rn["jax"]
    concat_in = [
        jax.device_put(
            np.concatenate([np.asarray(in_maps[c][name])
                            for c in range(NCORES)], axis=0), rn["sharding"])
        for name in rn["in_names"]
    ]
    concat_zero = [
        jax.device_put(
            np.zeros((NCORES * z.shape[0], *z.shape[1:]), z.dtype),
            rn["sharding"])
        for z in rn["zero_outs"]
    ]
    out = rn["fn"](*concat_in, *concat_zero)
    jax.block_until_ready(out)

    bench = int(os.environ.get("BASS_KERNEL_BENCH", "0"))
    if bench > 0:
        # warm
        for _ in range(2):
            jax.block_until_ready(rn["fn"](*concat_in, *concat_zero))
        t0 = _time.perf_counter()
        for _ in range(bench):
            o = rn["fn"](*concat_in, *concat_zero)
        jax.block_until_ready(o)
        dt = (_time.perf_counter() - t0) / bench
        LAST_PROFILE["bench_ns"] = dt * 1e9
        print(f"[kernel] bench: {bench} iters, {dt*1e6:.1f} us/iter")

    results = []
    for c in range(NCORES):
        m = {}
        for i, name in enumerate(rn["out_names"]):
            a = np.asarray(out[i])
            per = a.shape[0] // NCORES
            m[name] = a[c * per:(c + 1) * per]
        results.append(m)
    return results


def _host_consts(ob, db, s):
    """a = |s|*sqrt(0.5) is folded into the distance side so that the device
    computes error_distance as -(a*(g-w))^2 with no explicit scale op."""
    a = np.sqrt(0.5) * abs(float(s))
    p = np.arange(128, dtype=np.int64)[:, None]
    v = np.arange(BANDW, dtype=np.int64)[None, :]
    u = v - 896 - p          # j - i
    band_g = (a * np.log(np.abs(u).astype(np.float64) + 1.0)).astype(np.float32)
    band_l = np.tril(np.ones((128, 128), np.float32))
    bias2 = np.array([[ob, a * db]], np.float32)
    return band_g, band_l, bias2, a


def _numpy_ref(inputs):
    """Exact fallback (handles a non-zero attention_mask, never expected)."""
    x = np.asarray(inputs["input_tensor"], np.float64)
    mask = np.asarray(inputs["attention_mask"], np.float64)
    Wq = np.asarray(inputs["Wq"], np.float64)
    Wk = np.asarray(inputs["Wk"], np.float64)
    Wv = np.asarray(inputs["Wv"], np.float64)
    bq = np.asarray(inputs["bq"], np.float64)
    bk = np.asarray(inputs["bk"], np.float64)
    bv = np.asarray(inputs["bv"], np.float64)
    ow = np.asarray(inputs["order_w"], np.float64)
    ob = float(np.asarray(inputs["order_b"]))
    dw = np.asarray(inputs["dist_w"], np.float64)
    db = float(np.asarray(inputs["dist_b"]))
    s = float(np.asarray(inputs["scalar"]).reshape(-1)[0])

    mq = x @ Wq + bq
    mk = x @ Wk + bk
    mv = x @ Wv + bv

    def heads(a):
        return a.reshape(B, S, H, D).transpose(0, 2, 1, 3)

    qh, kh, vh = heads(mq), heads(mk), heads(mv)
    scores = np.einsum("bhid,bhjd->bhij", qh, kh)
    qo = qh @ ow[:D] + ob
    ko = kh @ ow[D:]
    z = qo[..., :, None] + ko[..., None, :]
    pr = 1.0 / (1.0 + np.exp(-z))
    tri = np.triu(np.ones((S, S)), k=1)
    eo = np.log(pr + 1e-24) * tri + np.log(1.0 - pr + 1e-24) * (1.0 - tri)
    idx = np.arange(S)
    g = np.log(np.abs(idx[None, :] - idx[:, None]) + 1.0)
    w = (qh @ dw[:D] + db)[..., :, None] + (kh @ dw[D:])[..., None, :]
    ed = -0.5 * s * s * np.square(g - w)
    adj = scores + eo + ed

    def softmax(sc):
        sc = sc / 8.0 + mask
        sc = sc - sc.max(axis=-1, keepdims=True)
        e = np.exp(sc)
        return e / e.sum(axis=-1, keepdims=True)

    return (mq.astype(np.float32), mk.astype(np.float32),
            vh.astype(np.float32), softmax(adj).astype(np.float32),
            softmax(scores).astype(np.float32))


def kernel(**inputs):
    x = np.ascontiguousarray(np.asarray(inputs["input_tensor"], np.float32))
    mask = np.asarray(inputs["attention_mask"], np.float32)
    if mask.any():
        return _numpy_ref(inputs)

    Wq = np.asarray(inputs["Wq"], np.float32)
    Wk = np.asarray(inputs["Wk"], np.float32)
    Wv = np.asarray(inputs["Wv"], np.float32)
    bq = np.asarray(inputs["bq"], np.float32)
    bk = np.asarray(inputs["bk"], np.float32)
    bv = np.asarray(inputs["bv"], np.float32)
    ow = np.asarray(inputs["order_w"], np.float32)
    ob = float(np.asarray(inputs["order_b"]))
    dw = np.asarray(inputs["dist_w"], np.float32)
    db = float(np.asarray(inputs["dist_b"]))
    s = float(np.asarray(inputs["scalar"]).reshape(-1)[0])

    band_g, band_l, bias2, a = _host_consts(ob, db, s)
    half = np.stack([ow[:D], a * dw[:D]], axis=1).astype(np.float32)  # [64,2]
    qswB = np.concatenate([half, half], axis=0)                       # [128,2]
    halfk = np.stack([ow[D:], a * dw[D:]], axis=1).astype(np.float32)
    kswB = np.concatenate([halfk, halfk], axis=0)

    in_maps = []
    for c in range(NCORES):
        b, hp_i = divmod(c, 4)
        c0 = hp_i * CPW
        csl = slice(c0, c0 + CPW)
        in_maps.append({
            "x": np.ascontiguousarray(x[b]),
            "wq": np.ascontiguousarray(Wq[:, csl]),
            "wk": np.ascontiguousarray(Wk[:, csl]),
            "wv": np.ascontiguousarray(Wv[:, csl]),
            "bqkv": np.ascontiguousarray(np.concatenate(
                [bq[csl], bk[csl], bv[csl]])[None, :]),
            "qsw": qswB, "ksw": kswB,
            "bandg": band_g, "bandl": band_l,
            "bias2": bias2,
        })

    results = _run_pjrt(in_maps)

    mixed_q = np.empty((B, S, HID), np.float32)
    mixed_k = np.empty((B, S, HID), np.float32)
    vh = np.empty((B, H, S, D), np.float32)
    pa = np.empty((B, H, S, S), np.float32)
    po = np.empty((B, H, S, S), np.float32)
    for c in range(NCORES):
        b, hp_i = divmod(c, 4)
        c0 = hp_i * CPW
        out = results[c]
        mixed_q[b][:, c0:c0 + CPW] = out["mq"]
        mixed_k[b][:, c0:c0 + CPW] = out["mk"]
        for j in range(HPC):
            vh[b, HPC * hp_i + j] = out["vo"][:, D * j:D * (j + 1)]
            pa[b, HPC * hp_i + j] = out["pa"][j]
            po[b, HPC * hp_i + j] = out["po"][j]
    return (mixed_q, mixed_k, vh, pa, po)
